# revision 19
# baseline (speedup 1.0000x reference)
"""Trainium2 Bass kernel for nn_DLCF_DCA (scatter_memory).

Reference computation, per sample b (B=128, S=256, H=768, K=64):
  keep_dep[s]  = (s==0) or any_k(depend[b,k] == s-1)
  keep_dpd[s]  = (s==0) or any_k(depended[b,k] == s-1)
  mult[s]      = w2 if s-1 in depended else (w1 if s-1 in depend else 0);
                 0 if s-1 in no_connect; 1 if s==0
  y1 = x * keep_dep;  y2 = x * keep_dpd;  y3 = x * mult

Strategy: pure data parallel over batch (16 samples per core, 8 cores).
Each core streams its [4096, 768] shard with 32 consecutive token-rows per
SBUF partition so every DMA moves long contiguous chunks per partition.
Multiplier masks are built in the matching [partition, row-in-partition]
layout: each index k is decomposed as (q, r) = divmod(b*256 + idx + 1, 32)
and membership counts come from one-hot compares contracted on the tensor
engine (count[p, r] = sum_k Q[k,p] * R[k,r]). The bulk work is then three
per-partition-scalar multiplies per 768-wide row block (vector + scalar
engines) between streamed input and output DMAs.
"""

import contextlib
import os
import sys

import numpy as np

if "/opt/trn_rl_repo" not in sys.path:
    sys.path.insert(0, "/opt/trn_rl_repo")

N_CORES = 8
B, S, H, K = 128, 256, 768, 64
BL = B // N_CORES          # samples per core
ROWS = BL * S              # 4096 token-rows per core
RPP = ROWS // 128          # 32 consecutive rows per partition
ND = 16                    # DMA tiles over the free dim
RPT = RPP // ND            # 4 row-blocks per tile
NCHUNK = BL * K // 128     # 8 contraction chunks for membership counts

_cache = {}


def _split_multiwaits(nc, max_waits=1):
    """walrus in this container only accepts one sync-wait per instruction;
    splice extra waits onto single-wait NoOps just before the offender."""
    from concourse import mybir

    n = 0
    for func in nc.m.functions:
        for bb in func.blocks:
            insts = bb.instructions
            i = 0
            while i < len(insts):
                ins = insts[i]
                si = getattr(ins, "sync_info", None)
                if si is None or len(si.on_wait) <= max_waits:
                    i += 1
                    continue
                waits = list(si.on_wait)
                keep = waits[-max_waits:]
                extra = waits[:-max_waits]
                nops = []
                for j in range(0, len(extra), max_waits):
                    n += 1
                    nops.append(
                        mybir.InstNoOp(
                            name=f"{ins.name}-ws{n}",
                            sync_info=mybir.SyncInfo(
                                on_wait=extra[j : j + max_waits], on_update=[]
                            ),
                            bass_nofuse=True,
                            engine=ins.engine,
                            ins=[],
                            outs=[],
                        )
                    )
                si.on_wait = keep
                for k, nop in enumerate(nops):
                    insts.insert(i + k, nop)
                i += len(nops) + 1
    return n


def _build():
    import concourse.bass as bass
    import concourse.tile as tile
    from concourse import mybir

    f32 = mybir.dt.float32
    bf16 = mybir.dt.bfloat16
    i32 = mybir.dt.int32
    eq = mybir.AluOpType.is_equal
    nc = bass.Bass()

    x = nc.dram_tensor("x", [ROWS, H], bf16, kind="ExternalInput")
    # meta[p, :]: q/r per list ([p, chunk]), p0q, p0r, w1p, w2p, iota, iota2
    NMETA = 6 * NCHUNK + 4 + 128 + RPP
    meta = nc.dram_tensor("meta", [128 * NMETA], f32, kind="ExternalInput")
    ys = [nc.dram_tensor(f"y{i}", [ROWS, H], bf16, kind="ExternalOutput")
          for i in (1, 2, 3)]

    NDO = 8                     # output DMA tiles (6KB per partition each)
    RPTO = RPP // NDO           # 4 row-blocks per output tile

    with tile.TileContext(nc) as tc, contextlib.ExitStack() as ctx:
        const = ctx.enter_context(tc.tile_pool(name="const", bufs=1))
        psum = ctx.enter_context(tc.tile_pool(name="psum", bufs=1, space="PSUM"))
        xpool = ctx.enter_context(tc.tile_pool(name="xpool", bufs=ND))
        ypool = ctx.enter_context(tc.tile_pool(name="ypool", bufs=4))

        # --- small loads / iota -----------------------------------------
        mt = const.tile([128, NMETA], f32, name="meta")
        nc.scalar.dma_start(out=mt[:], in_=meta.rearrange("(p c) -> p c", p=128))
        qT = {n: mt[:, i * NCHUNK : (i + 1) * NCHUNK]
              for i, n in enumerate(("dep", "dpd", "noc"))}
        rT = {n: mt[:, (3 + i) * NCHUNK : (4 + i) * NCHUNK]
              for i, n in enumerate(("dep", "dpd", "noc"))}
        p0qT = mt[:, 6 * NCHUNK : 6 * NCHUNK + 1]
        p0rT = mt[:, 6 * NCHUNK + 1 : 6 * NCHUNK + 2]
        w1T = mt[:, 6 * NCHUNK + 2 : 6 * NCHUNK + 3]
        w2T = mt[:, 6 * NCHUNK + 3 : 6 * NCHUNK + 4]

        iota = mt[:, 6 * NCHUNK + 4 : 6 * NCHUNK + 4 + 128]
        iota2 = mt[:, 6 * NCHUNK + 4 + 128 : 6 * NCHUNK + 4 + 128 + RPP]

        # --- all x tiles stream in on the ACT HWDGE ring -----------------
        NDR = ND                     # read tile granularity
        RPR = RPP // NDR             # row-blocks per read tile
        xrr = x.rearrange("(p d q) h -> d p (q h)", p=128, d=NDR)
        yr = [y.rearrange("(p d q) h -> d p (q h)", p=128, d=NDO) for y in ys]
        xts = {}
        for d in range(NDR):
            xts[d] = xpool.tile([128, RPR * H], bf16, name="xt")
            nc.sync.dma_start(out=xts[d][:], in_=xrr[d])

        mul = mybir.AluOpType.mult
        add = mybir.AluOpType.add
        mn = mybir.AluOpType.min
        mx = mybir.AluOpType.max

        # --- all membership one-hots up front (bf16, perf-mode matmuls) --
        def onehots(qcol, rcol, name):
            qt = const.tile([128, 128], bf16, name=f"Q{name}")
            nc.vector.tensor_scalar(qt[:], iota[:], qcol, None, op0=eq)
            rt = const.tile([128, RPP], bf16, name=f"R{name}")
            nc.vector.tensor_scalar(rt[:], iota2[:], rcol, None, op0=eq)
            return qt, rt

        def cols(ap):
            return [ap[:, c : c + 1] for c in range(NCHUNK)]

        oh = {"p0": [onehots(p0qT, p0rT, "p0")]}
        for n in ("dep", "dpd", "noc"):
            oh[n] = [onehots(q, r, f"{n}{c}")
                     for c, (q, r) in enumerate(zip(cols(qT[n]), cols(rT[n])))]

        # --- membership counts: pure PE chains, queued early --------------
        def count_list(name):
            pct = psum.tile([128, RPP], f32, name=f"pc_{name}")
            hs = oh[name]
            for c, (qt, rt) in enumerate(hs):
                nc.tensor.matmul(pct[:], lhsT=qt[:], rhs=rt[:],
                                 start=(c == 0), stop=(c == len(hs) - 1))
            return pct

        pc_p0 = count_list("p0")
        pc_dep = count_list("dep")
        pc_dpd = count_list("dpd")
        pc_noc = count_list("noc")

        p0m = const.tile([128, RPP], f32)
        nc.vector.tensor_copy(p0m[:], pc_p0[:])  # already 0/1
        dep1 = const.tile([128, RPP], f32)
        nc.vector.tensor_scalar(dep1[:], pc_dep[:], 1.0, None, op0=mn)
        m1 = const.tile([128, RPP], f32)
        nc.vector.tensor_tensor(m1[:], dep1[:], p0m[:], op=mx)

        dpd1 = const.tile([128, RPP], f32)
        nc.vector.tensor_scalar(dpd1[:], pc_dpd[:], 1.0, None, op0=mn)
        m2 = const.tile([128, RPP], f32)
        nc.vector.tensor_tensor(m2[:], dpd1[:], p0m[:], op=mx)

        # m3 = ((dep1*w1)*(1-dpd1) + dpd1*w2) * (1-noc1); then 1 at s==0
        m3 = const.tile([128, RPP], f32)
        inv = const.tile([128, RPP], f32)
        tmp = const.tile([128, RPP], f32)
        nc.vector.tensor_scalar(m3[:], dep1[:], w1T, None, op0=mul)
        nc.vector.tensor_scalar(inv[:], dpd1[:], -1.0, 1.0, op0=mul, op1=add)
        nc.vector.tensor_tensor(m3[:], m3[:], inv[:], op=mul)
        nc.vector.tensor_scalar(tmp[:], dpd1[:], w2T, None, op0=mul)
        nc.vector.tensor_tensor(m3[:], m3[:], tmp[:], op=add)
        nc.vector.tensor_scalar(inv[:], pc_noc[:], 1.0, None, op0=mn)  # noc1
        nc.vector.tensor_scalar(inv[:], inv[:], -1.0, 1.0, op0=mul, op1=add)
        nc.vector.tensor_tensor(m3[:], m3[:], inv[:], op=mul)
        nc.vector.tensor_scalar(inv[:], p0m[:], -1.0, 1.0, op0=mul, op1=add)
        nc.vector.tensor_tensor(m3[:], m3[:], inv[:], op=mul)
        nc.vector.tensor_tensor(m3[:], m3[:], p0m[:], op=add)

        # multiply/write helpers: one wide broadcast multiply per tile on
        # DVE; per-block activation copies on ACT for part of y3.
        mb = {}
        for name, mk in (("m1", m1), ("m2", m2), ("m3", m3)):
            t = const.tile([128, RPP], bf16, name=f"{name}b")
            nc.vector.tensor_copy(t[:], mk[:])
            mb[name] = t

        def xsrc(r):
            return xts[r // RPR][:, (r % RPR) * H : (r % RPR + 1) * H]

        def xsrc_wide(d):
            # RPTO row-blocks of tile d live in RPTO//RPR consecutive x tiles
            base = d * RPTO // RPR
            return [xts[base + i] for i in range(RPTO // RPR)]

        def emit_dve(yi, mname, d, ring):
            yt = ypool.tile([128, RPTO, H], bf16, name=f"y{yi}t")
            msk = mb[mname][:, d * RPTO : (d + 1) * RPTO, None]
            srcs = xsrc_wide(d)
            nper = RPTO // len(srcs)
            for i, xt in enumerate(srcs):
                nc.vector.tensor_tensor(
                    yt[:, i * nper : (i + 1) * nper],
                    xt[:].rearrange("p (r h) -> p r h", h=H),
                    msk[:, i * nper : (i + 1) * nper].to_broadcast(
                        [128, nper, H]
                    ),
                    op=mul,
                )
            ring.dma_start(out=yr[yi - 1][d], in_=yt[:].rearrange("p r h -> p (r h)"))

        def emit_act(yi, mk, d, ring):
            yt = ypool.tile([128, RPTO * H], bf16, name=f"y{yi}ta")
            for g in range(RPTO):
                r = d * RPTO + g
                blk = slice(g * H, (g + 1) * H)
                nc.scalar.activation(
                    yt[:, blk], xsrc(r),
                    mybir.ActivationFunctionType.Copy,
                    scale=mk[:, r : r + 1],
                )
            ring.dma_start(out=yr[yi - 1][d], in_=yt[:])

        for d in range(NDO):
            emit_dve(1, "m1", d, nc.gpsimd)
            if d % 2:
                emit_dve(3, "m3", d, nc.gpsimd)
            else:
                emit_act(3, m3, d, nc.gpsimd)
            emit_dve(2, "m2", d, nc.sync)

    _split_multiwaits(nc)
    return nc


def _prep_inputs(bert_local_out, depend, depended, no_connect,
                 depend_weight, depended_weight):
    import ml_dtypes

    x = np.ascontiguousarray(
        np.asarray(bert_local_out, dtype=np.float32).astype(ml_dtypes.bfloat16)
    )
    idx = {
        "dep": np.asarray(depend, dtype=np.int64),
        "dpd": np.asarray(depended, dtype=np.int64),
        "noc": np.asarray(no_connect, dtype=np.int64),
    }
    w1 = np.asarray(depend_weight, dtype=np.float32)
    w2 = np.asarray(depended_weight, dtype=np.float32)

    p0q = np.full(128, 9999.0, dtype=np.float32)
    p0r = np.full(128, 9999.0, dtype=np.float32)
    p0q[:BL] = 8 * np.arange(BL)
    p0r[:BL] = 0.0

    pidx = np.arange(128) // (128 // BL)  # sample owning each partition
    boff = np.arange(BL, dtype=np.int64)[:, None] * S  # b*256
    NMETA = 6 * NCHUNK + 4 + 128 + RPP

    in_maps = []
    for c in range(N_CORES):
        sl = slice(c * BL, (c + 1) * BL)
        meta = np.empty((128, NMETA), dtype=np.float32)
        for i, n in enumerate(("dep", "dpd", "noc")):
            g = (idx[n][sl] + boff + 1).reshape(-1)  # global position + 1
            meta[:, i * NCHUNK : (i + 1) * NCHUNK] = (
                (g // RPP).astype(np.float32).reshape(NCHUNK, 128).T
            )
            meta[:, (3 + i) * NCHUNK : (4 + i) * NCHUNK] = (
                (g % RPP).astype(np.float32).reshape(NCHUNK, 128).T
            )
        meta[:, 6 * NCHUNK] = p0q
        meta[:, 6 * NCHUNK + 1] = p0r
        meta[:, 6 * NCHUNK + 2] = w1[sl][pidx]
        meta[:, 6 * NCHUNK + 3] = w2[sl][pidx]
        meta[:, 6 * NCHUNK + 4 : 6 * NCHUNK + 4 + 128] = np.arange(128)[None, :]
        meta[:, 6 * NCHUNK + 4 + 128 :] = np.arange(RPP)[None, :]
        in_maps.append({
            "x": x[sl].reshape(ROWS, H),
            "meta": np.ascontiguousarray(meta).reshape(-1),
        })
    return in_maps


def kernel(bert_local_out, depend, depended, no_connect,
           depend_weight, depended_weight):
    from concourse.bass_utils import run_bass_kernel_spmd

    if "nc" not in _cache:
        _cache["nc"] = _build()
    nc = _cache["nc"]

    in_maps = _prep_inputs(bert_local_out, depend, depended, no_connect,
                           depend_weight, depended_weight)

    pdir = os.environ.get("KERNEL_PROFILE_DIR")
    ctx = contextlib.nullcontext()
    if pdir:
        import concourse.bass2jax as b2j
        from trn_agent_boot.trn_boot import _ntff_profile_via_ctypes

        if not getattr(b2j, "_neff_capture_patched", False):
            orig = b2j.rename_neff_tensors_and_patch_header

            def patched(neff_path, mapping):
                data = orig(neff_path, mapping)
                cap = os.environ.get("KERNEL_PROFILE_DIR")
                if cap:
                    os.makedirs(cap, exist_ok=True)
                    with open(os.path.join(cap, "model.neff"), "wb") as f:
                        f.write(data)
                return data

            b2j.rename_neff_tensors_and_patch_header = patched
            b2j._neff_capture_patched = True
        os.makedirs(pdir, exist_ok=True)
        hookf = _ntff_profile_via_ctypes("/opt/axon/libaxon_pjrt.so")
        if hookf is not None:
            dev = None if os.environ.get("KERNEL_PROFILE_ALL") else [0]
            ctx = hookf(pdir, dev)

    with ctx:
        res = run_bass_kernel_spmd(nc, in_maps, list(range(N_CORES)))

    outs = []
    for name in ("y1", "y2", "y3"):
        full = np.empty((B, S, H), dtype=np.float32)
        for c in range(N_CORES):
            full[c * BL : (c + 1) * BL] = (
                res.results[c][name].astype(np.float32).reshape(BL, S, H)
            )
        outs.append(full)
    return tuple(outs)



# revision 20
# speedup vs baseline: 1.1864x; 1.1864x over previous
"""Trainium2 Bass kernel for nn_DLCF_DCA (scatter_memory).

Reference computation, per sample b (B=128, S=256, H=768, K=64):
  keep_dep[s]  = (s==0) or any_k(depend[b,k] == s-1)
  keep_dpd[s]  = (s==0) or any_k(depended[b,k] == s-1)
  mult[s]      = w2 if s-1 in depended else (w1 if s-1 in depend else 0);
                 0 if s-1 in no_connect; 1 if s==0
  y1 = x * keep_dep;  y2 = x * keep_dpd;  y3 = x * mult

Strategy: pure data parallel over batch (16 samples per core, 8 cores).
Each core streams its [4096, 768] shard with 32 consecutive token-rows per
SBUF partition so every DMA moves long contiguous chunks per partition.
Multiplier masks are built in the matching [partition, row-in-partition]
layout: each index k is decomposed as (q, r) = divmod(b*256 + idx + 1, 32)
and membership counts come from one-hot compares contracted on the tensor
engine (count[p, r] = sum_k Q[k,p] * R[k,r]). The bulk work is then three
per-partition-scalar multiplies per 768-wide row block (vector + scalar
engines) between streamed input and output DMAs.
"""

import contextlib
import os
import sys

import numpy as np

if "/opt/trn_rl_repo" not in sys.path:
    sys.path.insert(0, "/opt/trn_rl_repo")

N_CORES = 8
B, S, H, K = 128, 256, 768, 64
BL = B // N_CORES          # samples per core
ROWS = BL * S              # 4096 token-rows per core
RPP = ROWS // 128          # 32 consecutive rows per partition
ND = 16                    # DMA tiles over the free dim
RPT = RPP // ND            # 4 row-blocks per tile
NCHUNK = BL * K // 128     # 8 contraction chunks for membership counts

_cache = {}


def _split_multiwaits(nc, max_waits=1):
    """walrus in this container only accepts one sync-wait per instruction;
    splice extra waits onto single-wait NoOps just before the offender."""
    from concourse import mybir

    n = 0
    for func in nc.m.functions:
        for bb in func.blocks:
            insts = bb.instructions
            i = 0
            while i < len(insts):
                ins = insts[i]
                si = getattr(ins, "sync_info", None)
                if si is None or len(si.on_wait) <= max_waits:
                    i += 1
                    continue
                waits = list(si.on_wait)
                keep = waits[-max_waits:]
                extra = waits[:-max_waits]
                nops = []
                for j in range(0, len(extra), max_waits):
                    n += 1
                    nops.append(
                        mybir.InstNoOp(
                            name=f"{ins.name}-ws{n}",
                            sync_info=mybir.SyncInfo(
                                on_wait=extra[j : j + max_waits], on_update=[]
                            ),
                            bass_nofuse=True,
                            engine=ins.engine,
                            ins=[],
                            outs=[],
                        )
                    )
                si.on_wait = keep
                for k, nop in enumerate(nops):
                    insts.insert(i + k, nop)
                i += len(nops) + 1
    return n


def _build():
    import concourse.bass as bass
    import concourse.tile as tile
    from concourse import mybir

    f32 = mybir.dt.float32
    bf16 = mybir.dt.bfloat16
    i32 = mybir.dt.int32
    eq = mybir.AluOpType.is_equal
    nc = bass.Bass()

    x = nc.dram_tensor("x", [ROWS, H], bf16, kind="ExternalInput")
    # meta[p, :]: q/r per list ([p, chunk]), p0q, p0r, w1p, w2p, iota, iota2
    NMETA = 6 * NCHUNK + 4 + 128 + RPP
    meta = nc.dram_tensor("meta", [128 * NMETA], f32, kind="ExternalInput")
    ys = [nc.dram_tensor(f"y{i}", [ROWS, H], bf16, kind="ExternalOutput")
          for i in (1, 2, 3)]

    NDO = 8                     # output DMA tiles (6KB per partition each)
    RPTO = RPP // NDO           # 4 row-blocks per output tile

    with tile.TileContext(nc) as tc, contextlib.ExitStack() as ctx:
        const = ctx.enter_context(tc.tile_pool(name="const", bufs=1))
        psum = ctx.enter_context(tc.tile_pool(name="psum", bufs=1, space="PSUM"))
        xpool = ctx.enter_context(tc.tile_pool(name="xpool", bufs=ND))
        ypool = ctx.enter_context(tc.tile_pool(name="ypool", bufs=4))

        # --- small loads / iota -----------------------------------------
        mt = const.tile([128, NMETA], f32, name="meta")
        nc.scalar.dma_start(out=mt[:], in_=meta.rearrange("(p c) -> p c", p=128))
        qT = {n: mt[:, i * NCHUNK : (i + 1) * NCHUNK]
              for i, n in enumerate(("dep", "dpd", "noc"))}
        rT = {n: mt[:, (3 + i) * NCHUNK : (4 + i) * NCHUNK]
              for i, n in enumerate(("dep", "dpd", "noc"))}
        p0qT = mt[:, 6 * NCHUNK : 6 * NCHUNK + 1]
        p0rT = mt[:, 6 * NCHUNK + 1 : 6 * NCHUNK + 2]
        w1T = mt[:, 6 * NCHUNK + 2 : 6 * NCHUNK + 3]
        w2T = mt[:, 6 * NCHUNK + 3 : 6 * NCHUNK + 4]

        iota = mt[:, 6 * NCHUNK + 4 : 6 * NCHUNK + 4 + 128]
        iota2 = mt[:, 6 * NCHUNK + 4 + 128 : 6 * NCHUNK + 4 + 128 + RPP]

        # --- all x tiles stream in on the ACT HWDGE ring -----------------
        NDR = ND                     # read tile granularity
        RPR = RPP // NDR             # row-blocks per read tile
        xrr = x.rearrange("(p d q) h -> d p (q h)", p=128, d=NDR)
        yr = [y.rearrange("(p d q) h -> d p (q h)", p=128, d=NDO) for y in ys]
        xts = {}
        for d in range(NDR):
            xts[d] = xpool.tile([128, RPR * H], bf16, name="xt")
            nc.sync.dma_start(out=xts[d][:], in_=xrr[d])

        mul = mybir.AluOpType.mult
        add = mybir.AluOpType.add
        mn = mybir.AluOpType.min
        mx = mybir.AluOpType.max

        # --- all membership one-hots up front (bf16, perf-mode matmuls) --
        def onehots(qcol, rcol, name):
            qt = const.tile([128, 128], bf16, name=f"Q{name}")
            nc.vector.tensor_scalar(qt[:], iota[:], qcol, None, op0=eq)
            rt = const.tile([128, RPP], bf16, name=f"R{name}")
            nc.vector.tensor_scalar(rt[:], iota2[:], rcol, None, op0=eq)
            return qt, rt

        def cols(ap):
            return [ap[:, c : c + 1] for c in range(NCHUNK)]

        oh = {"p0": [onehots(p0qT, p0rT, "p0")]}
        for n in ("dep", "dpd", "noc"):
            oh[n] = [onehots(q, r, f"{n}{c}")
                     for c, (q, r) in enumerate(zip(cols(qT[n]), cols(rT[n])))]

        # --- membership counts: pure PE chains, queued early --------------
        def count_list(name):
            pct = psum.tile([128, RPP], f32, name=f"pc_{name}")
            hs = oh[name]
            for c, (qt, rt) in enumerate(hs):
                nc.tensor.matmul(pct[:], lhsT=qt[:], rhs=rt[:],
                                 start=(c == 0), stop=(c == len(hs) - 1))
            return pct

        pc_p0 = count_list("p0")
        pc_dep = count_list("dep")
        pc_dpd = count_list("dpd")
        pc_noc = count_list("noc")

        p0m = const.tile([128, RPP], f32)
        nc.vector.tensor_copy(p0m[:], pc_p0[:])  # already 0/1
        dep1 = const.tile([128, RPP], f32)
        nc.vector.tensor_scalar(dep1[:], pc_dep[:], 1.0, None, op0=mn)
        m1 = const.tile([128, RPP], f32)
        nc.vector.tensor_tensor(m1[:], dep1[:], p0m[:], op=mx)

        dpd1 = const.tile([128, RPP], f32)
        nc.vector.tensor_scalar(dpd1[:], pc_dpd[:], 1.0, None, op0=mn)
        m2 = const.tile([128, RPP], f32)
        nc.vector.tensor_tensor(m2[:], dpd1[:], p0m[:], op=mx)

        # m3 = ((dep1*w1)*(1-dpd1) + dpd1*w2) * (1-noc1); then 1 at s==0
        m3 = const.tile([128, RPP], f32)
        inv = const.tile([128, RPP], f32)
        tmp = const.tile([128, RPP], f32)
        nc.vector.tensor_scalar(m3[:], dep1[:], w1T, None, op0=mul)
        nc.vector.tensor_scalar(inv[:], dpd1[:], -1.0, 1.0, op0=mul, op1=add)
        nc.vector.tensor_tensor(m3[:], m3[:], inv[:], op=mul)
        nc.vector.tensor_scalar(tmp[:], dpd1[:], w2T, None, op0=mul)
        nc.vector.tensor_tensor(m3[:], m3[:], tmp[:], op=add)
        nc.vector.tensor_scalar(inv[:], pc_noc[:], 1.0, None, op0=mn)  # noc1
        nc.vector.tensor_scalar(inv[:], inv[:], -1.0, 1.0, op0=mul, op1=add)
        nc.vector.tensor_tensor(m3[:], m3[:], inv[:], op=mul)
        nc.vector.tensor_scalar(inv[:], p0m[:], -1.0, 1.0, op0=mul, op1=add)
        nc.vector.tensor_tensor(m3[:], m3[:], inv[:], op=mul)
        nc.vector.tensor_tensor(m3[:], m3[:], p0m[:], op=add)

        # multiply/write helpers: per-block tensor_scalar on DVE (bf16 2x
        # rate); per-block activation copies on ACT for part of y3.
        def xsrc(r):
            return xts[r // RPR][:, (r % RPR) * H : (r % RPR + 1) * H]

        def emit_dve(yi, mk, d, ring):
            yt = ypool.tile([128, RPTO * H], bf16, name=f"y{yi}t")
            for g in range(RPTO):
                r = d * RPTO + g
                blk = slice(g * H, (g + 1) * H)
                nc.vector.tensor_scalar(
                    yt[:, blk], xsrc(r), mk[:, r : r + 1], None, op0=mul
                )
            ring.dma_start(out=yr[yi - 1][d], in_=yt[:])

        def emit_act(yi, mk, d, ring):
            yt = ypool.tile([128, RPTO * H], bf16, name=f"y{yi}ta")
            for g in range(RPTO):
                r = d * RPTO + g
                blk = slice(g * H, (g + 1) * H)
                nc.scalar.activation(
                    yt[:, blk], xsrc(r),
                    mybir.ActivationFunctionType.Copy,
                    scale=mk[:, r : r + 1],
                )
            ring.dma_start(out=yr[yi - 1][d], in_=yt[:])

        for d in range(NDO):
            emit_dve(1, m1, d, nc.gpsimd)
            if d % 2:
                emit_dve(3, m3, d, nc.gpsimd)
            else:
                emit_act(3, m3, d, nc.gpsimd)
            emit_dve(2, m2, d, nc.sync)

    _split_multiwaits(nc)
    return nc


def _prep_inputs(bert_local_out, depend, depended, no_connect,
                 depend_weight, depended_weight):
    import ml_dtypes

    x = np.ascontiguousarray(
        np.asarray(bert_local_out, dtype=np.float32).astype(ml_dtypes.bfloat16)
    )
    idx = {
        "dep": np.asarray(depend, dtype=np.int64),
        "dpd": np.asarray(depended, dtype=np.int64),
        "noc": np.asarray(no_connect, dtype=np.int64),
    }
    w1 = np.asarray(depend_weight, dtype=np.float32)
    w2 = np.asarray(depended_weight, dtype=np.float32)

    p0q = np.full(128, 9999.0, dtype=np.float32)
    p0r = np.full(128, 9999.0, dtype=np.float32)
    p0q[:BL] = 8 * np.arange(BL)
    p0r[:BL] = 0.0

    pidx = np.arange(128) // (128 // BL)  # sample owning each partition
    boff = np.arange(BL, dtype=np.int64)[:, None] * S  # b*256
    NMETA = 6 * NCHUNK + 4 + 128 + RPP

    in_maps = []
    for c in range(N_CORES):
        sl = slice(c * BL, (c + 1) * BL)
        meta = np.empty((128, NMETA), dtype=np.float32)
        for i, n in enumerate(("dep", "dpd", "noc")):
            g = (idx[n][sl] + boff + 1).reshape(-1)  # global position + 1
            meta[:, i * NCHUNK : (i + 1) * NCHUNK] = (
                (g // RPP).astype(np.float32).reshape(NCHUNK, 128).T
            )
            meta[:, (3 + i) * NCHUNK : (4 + i) * NCHUNK] = (
                (g % RPP).astype(np.float32).reshape(NCHUNK, 128).T
            )
        meta[:, 6 * NCHUNK] = p0q
        meta[:, 6 * NCHUNK + 1] = p0r
        meta[:, 6 * NCHUNK + 2] = w1[sl][pidx]
        meta[:, 6 * NCHUNK + 3] = w2[sl][pidx]
        meta[:, 6 * NCHUNK + 4 : 6 * NCHUNK + 4 + 128] = np.arange(128)[None, :]
        meta[:, 6 * NCHUNK + 4 + 128 :] = np.arange(RPP)[None, :]
        in_maps.append({
            "x": x[sl].reshape(ROWS, H),
            "meta": np.ascontiguousarray(meta).reshape(-1),
        })
    return in_maps


def kernel(bert_local_out, depend, depended, no_connect,
           depend_weight, depended_weight):
    from concourse.bass_utils import run_bass_kernel_spmd

    if "nc" not in _cache:
        _cache["nc"] = _build()
    nc = _cache["nc"]

    in_maps = _prep_inputs(bert_local_out, depend, depended, no_connect,
                           depend_weight, depended_weight)

    pdir = os.environ.get("KERNEL_PROFILE_DIR")
    ctx = contextlib.nullcontext()
    if pdir:
        import concourse.bass2jax as b2j
        from trn_agent_boot.trn_boot import _ntff_profile_via_ctypes

        if not getattr(b2j, "_neff_capture_patched", False):
            orig = b2j.rename_neff_tensors_and_patch_header

            def patched(neff_path, mapping):
                data = orig(neff_path, mapping)
                cap = os.environ.get("KERNEL_PROFILE_DIR")
                if cap:
                    os.makedirs(cap, exist_ok=True)
                    with open(os.path.join(cap, "model.neff"), "wb") as f:
                        f.write(data)
                return data

            b2j.rename_neff_tensors_and_patch_header = patched
            b2j._neff_capture_patched = True
        os.makedirs(pdir, exist_ok=True)
        hookf = _ntff_profile_via_ctypes("/opt/axon/libaxon_pjrt.so")
        if hookf is not None:
            dev = None if os.environ.get("KERNEL_PROFILE_ALL") else [0]
            ctx = hookf(pdir, dev)

    with ctx:
        res = run_bass_kernel_spmd(nc, in_maps, list(range(N_CORES)))

    outs = []
    for name in ("y1", "y2", "y3"):
        full = np.empty((B, S, H), dtype=np.float32)
        for c in range(N_CORES):
            full[c * BL : (c + 1) * BL] = (
                res.results[c][name].astype(np.float32).reshape(BL, S, H)
            )
        outs.append(full)
    return tuple(outs)

